# revision 14
# baseline (speedup 1.0000x reference)
"""Multi-head attention (B=384, S=128, E=512, H=4, D=128) on 8 TRN2 NeuronCores.

Data-parallel: batch 384 -> 48 per core, projection weights replicated.

v2: all matmuls in fp16 (1 cyc/row at any moving size, vs fp32r's ~2-4
cyc/row at moving=128), and both transposes moved off the PE onto the DMA
XBAR (InstDmaTransposeAnt, 16-bit dtypes, ~14ns per 16x128 tile):

  xT   = transposing DMA load   x[b] DRAM [S,E] fp16 -> xt [128, EC, S] SBUF
  QT   = Wq^T @ xT + bq         [E_out, rows] fp16 (lhsT = Wq chunk, rhs = xT)
  KT   = Wk^T @ xT + bk         [E_out, rows] fp16
  V    = x @ Wv + bv            [rows, E_out] fp16 (lhsT = xT chunk, rhs = Wv)
  per batch (4 heads packed along the PSUM free dim):
    S    = qT.T @ kT            [S, H, T] scores in PSUM fp32
    w    = exp(S) fp32 (no max-sub: |S| < 88 so fp32 exp cannot overflow)
    wn   = w * (1/rowsum)       normalize -> fp16 (Pool engine)
    wT   = transposing DMA      SBUF [S, H*T] -> [T, H, S] SBUF
    attT = lhsT(v) @ wT         [D, H, S] fp16 inputs, PSUM fp32
  O    = att @ Wo + bo          (lhsT = attT chunk, rhs = Wo) -> [rows, E] f32

Precision: fp16 inputs give ~3e-4 elementwise rounding; logit abs err
~8e-3 -> output rel err ~3-4e-3 (gate 2e-2). exp/softmax stats stay fp32.

Engine split per chunk (vs ~15.4us of PE matmul): ACT q/k bias-adds + exp
(~8us), DVE v bias + reduce_sum + recip (~6us), Pool normalize + attT
copies + o bias (~8us), SP issues all DMAs (~8us). Weights go out on the
ACT DMA queue at startup so they don't serialize behind x on SP.

Scheduling: engine streams execute in emission order, software pipeline:
scores(k) | xT-DMAs(k+2) | projections(k+1) | attention-tail(k). The wT
DMA for chunk k is issued during scores(k) and consumed by the tail after
proj(k+1), hiding ~1.4us of DMA latency; the last chunk transposes wT on
the PE instead (idle at drain). Dummy bf16 matmuls warm the PE HAM
clock-gate during the initial weight/x DMA window.
"""

import numpy as np

import concourse.bass as bass
import concourse.tile as tile
import concourse.mybir as mybir
from concourse import bacc
from concourse.bass_utils import run_bass_kernel_spmd
from concourse.masks import make_identity

B, S, E, H, D = 384, 128, 512, 4, 128
NCORES = 8
BLOC = B // NCORES  # 48 batches per core
NB = 4  # batches per chunk
NCHUNK = BLOC // NB
NBS = NB * S  # 512 rows of x per chunk
EC = E // 128  # 4 chunks of the embed dim

F32 = mybir.dt.float32
F16 = mybir.dt.float16
BF16 = mybir.dt.bfloat16

_CACHE = {}


def build():
    nc = bacc.Bacc("TRN2", target_bir_lowering=False, debug=False, num_devices=NCORES)

    x = nc.dram_tensor("x", [BLOC, S, E], F16, kind="ExternalInput").ap()
    wq = nc.dram_tensor("Wq", [E, E], F16, kind="ExternalInput").ap()
    wk = nc.dram_tensor("Wk", [E, E], F16, kind="ExternalInput").ap()
    wv = nc.dram_tensor("Wv", [E, E], F16, kind="ExternalInput").ap()
    wo = nc.dram_tensor("Wo", [E, E], F16, kind="ExternalInput").ap()
    bq = nc.dram_tensor("bq", [E], F32, kind="ExternalInput").ap()
    bk = nc.dram_tensor("bk", [E], F32, kind="ExternalInput").ap()
    bv = nc.dram_tensor("bv", [E], F32, kind="ExternalInput").ap()
    bo = nc.dram_tensor("bo", [E], F32, kind="ExternalInput").ap()
    out = nc.dram_tensor("out", [BLOC, S, E], F32, kind="ExternalOutput").ap()

    with tile.TileContext(nc) as tc:
        with (
            tc.tile_pool(name="singles", bufs=1) as singles,
            tc.tile_pool(name="xp", bufs=4) as xp,
            tc.tile_pool(name="qkv", bufs=3) as qkv,
            tc.tile_pool(name="attn", bufs=3) as attn,
            tc.tile_pool(name="wsm", bufs=4) as wsm,
            tc.tile_pool(name="stats", bufs=8) as stats,
            tc.tile_pool(name="ps", bufs=8, space="PSUM") as ps,
        ):
            # --- weights / biases / identities ---
            w_sb = {}
            w_dram = {"q": wq, "k": wk, "v": wv, "o": wo}
            for name in ("q", "k", "v", "o"):
                w_sb[name] = singles.tile([128, EC, E], F16, tag=f"w{name}", name=f"w{name}")

            def load_weight(name, engine=None):
                # Weight DMAs are spread across issue queues so no engine's
                # compute stream queues behind 16 big DMA issues: q/k go out
                # on the ACT hwdge queue (needed first, few issues), v/o on
                # the gpsimd SWDGE queue (Pool is idle during startup).
                eng = engine if engine is not None else nc.scalar
                for c in range(EC):
                    eng.dma_start(
                        out=w_sb[name][:, c, :],
                        in_=w_dram[name][c * 128 : (c + 1) * 128, :],
                    )

            bq_sb = singles.tile([128, EC], F32, tag="bq")
            bk_sb = singles.tile([128, EC], F32, tag="bk")
            bv_sb = singles.tile([128, E], F32, tag="bv")
            bo_sb = singles.tile([128, E], F32, tag="bo")

            def load_biases():
                for t, b in ((bq_sb, bq), (bk_sb, bk)):
                    nc.sync.dma_start(
                        out=t,
                        in_=bass.AP(tensor=b.tensor, offset=0, ap=[[1, 128], [128, EC]]),
                    )
                for t, b in ((bv_sb, bv), (bo_sb, bo)):
                    nc.sync.dma_start(
                        out=t,
                        in_=bass.AP(tensor=b.tensor, offset=0, ap=[[0, 128], [1, E]]),
                    )

            ident_f16 = singles.tile([128, 128], F16, tag="idf16")
            make_identity(nc, ident_f16[:])

            # Warm the PE HAM clock-gate during the initial weight/x DMA
            # window with dummy matmuls (PE would otherwise start cold at
            # half clock). Output is never read.
            dummy_bf = singles.tile([128, E], BF16, tag="dummy")
            nc.vector.memset(dummy_bf, 0.0)
            ident_bf = singles.tile([128, 128], BF16, tag="idb")
            make_identity(nc, ident_bf[:])
            warm_ps = ps.tile([128, E], F32, tag="ps", name="warm")
            for _ in range(36):
                nc.tensor.matmul(warm_ps, ident_bf[:], dummy_bf, start=True, stop=True)

            def load_xt(chunk):
                """One transposing DMA per chunk: the whole 4-batch block
                [NBS, E] (DRAM-contiguous) transposes into xt [128, EC, NBS]
                with xt[p, c, j*128+s] = x[b0+j][s, c*128+p]."""
                b0 = chunk * NB
                xt = xp.tile([128, EC, NBS], F16, tag="xt")
                nc.sync.dma_start(
                    out=xt,
                    in_=bass.AP(
                        tensor=x.tensor, offset=b0 * S * E, ap=[[E, NBS], [1, E]]
                    ),
                    transpose=True,
                )
                return xt

            def proj(xt):
                """QT/KT/V projections from xT (all fp16)."""
                qt, kt = [], []
                for h in range(H):
                    p = ps.tile([128, NBS], F32, tag="ps")
                    for c in range(EC):
                        nc.tensor.matmul(
                            p,
                            w_sb["q"][:, c, h * 128 : (h + 1) * 128],
                            xt[:, c, :],
                            start=(c == 0),
                            stop=(c == EC - 1),
                        )
                    t = qkv.tile([128, NBS], F16, tag=f"qt{h}")
                    nc.scalar.add(out=t, in_=p, add=bq_sb[:, h : h + 1])
                    qt.append(t)
                    p = ps.tile([128, NBS], F32, tag="ps")
                    for c in range(EC):
                        nc.tensor.matmul(
                            p,
                            w_sb["k"][:, c, h * 128 : (h + 1) * 128],
                            xt[:, c, :],
                            start=(c == 0),
                            stop=(c == EC - 1),
                        )
                    t = qkv.tile([128, NBS], F16, tag=f"kt{h}")
                    nc.scalar.add(out=t, in_=p, add=bk_sb[:, h : h + 1])
                    kt.append(t)
                v_sb = []
                for j in range(NB):
                    p = ps.tile([128, E], F32, tag="ps")
                    for c in range(EC):
                        nc.tensor.matmul(
                            p,
                            xt[:, c, j * 128 : (j + 1) * 128],
                            w_sb["v"][:, c, :],
                            start=(c == 0),
                            stop=(c == EC - 1),
                        )
                    t = qkv.tile([128, E], F16, tag=f"v{j}")
                    nc.vector.tensor_add(out=t, in0=p, in1=bv_sb)
                    v_sb.append(t)
                return qt, kt, v_sb

            def attn_scores(qt, kt, pe_wt=False):
                """scores + softmax (no max-sub, fp32 exp/stats) -> fp16
                normalized w, transposed to wT in ONE chunk-wide DMA XBAR
                transpose (or on the PE for the drain chunk)."""
                w_f16 = wsm.tile([128, NB, H, 128], F16, tag="wf16")
                wt = wsm.tile([128, NB, H, 128], F16, tag="wt", name="wt")
                for j in range(NB):
                    ps_s = ps.tile([128, H, 128], F32, tag="ps")
                    for h in range(H):
                        nc.tensor.matmul(
                            ps_s[:, h, :],
                            qt[h][:, j * 128 : (j + 1) * 128],
                            kt[h][:, j * 128 : (j + 1) * 128],
                            start=True,
                            stop=True,
                        )
                    w_exp = wsm.tile([128, H, 128], F32, tag="wexp")
                    nc.scalar.activation(
                        out=w_exp,
                        in_=ps_s,
                        func=mybir.ActivationFunctionType.Exp,
                        bias=0.0,
                        scale=1.0,
                    )
                    sumexp = stats.tile([128, H], F32, tag="sumexp")
                    nc.vector.reduce_sum(
                        out=sumexp, in_=w_exp, axis=mybir.AxisListType.X
                    )
                    recip = stats.tile([128, H], F32, tag="recip")
                    nc.vector.reciprocal(out=recip, in_=sumexp)
                    for h in range(H):
                        nc.vector.tensor_scalar_mul(
                            out=w_f16[:, j, h, :],
                            in0=w_exp[:, h, :],
                            scalar1=recip[:, h : h + 1],
                        )
                if pe_wt:
                    for j in range(NB):
                        ps_wt = ps.tile([128, H, 128], F16, tag="ps")
                        for h in range(H):
                            nc.tensor.transpose(
                                ps_wt[:, h, :], w_f16[:, j, h, :], ident_f16[:]
                            )
                        nc.vector.tensor_copy(out=wt[:, j], in_=ps_wt)
                else:
                    nc.sync.dma_start(out=wt, in_=w_f16, transpose=True)
                return wt

            def attn_tail(chunk, wt, v_sb):
                """att = v.T-form matmuls against wT, O projection, store."""
                b0 = chunk * NB
                ats = []
                for j in range(NB):
                    ps_at = ps.tile([128, H, 128], F32, tag="ps")
                    for h in range(H):
                        nc.tensor.matmul(
                            ps_at[:, h, :],
                            v_sb[j][:, h * 128 : (h + 1) * 128],
                            wt[:, j, h, :],
                            start=True,
                            stop=True,
                        )
                    at = attn.tile([128, H, 128], F16, tag=f"at{j}")
                    if j % 2 == 0:
                        nc.scalar.copy(out=at, in_=ps_at)
                    else:
                        nc.vector.tensor_copy(out=at, in_=ps_at)
                    ats.append(at)
                for j in range(NB):
                    p = ps.tile([128, E], F32, tag="ps")
                    for h in range(H):
                        nc.tensor.matmul(
                            p,
                            ats[j][:, h, :],
                            w_sb["o"][:, h, :],
                            start=(h == 0),
                            stop=(h == H - 1),
                        )
                    o_sb = attn.tile([128, E], F32, tag=f"o{j}")
                    nc.vector.tensor_add(out=o_sb, in0=p, in1=bo_sb)
                    # Split each 256KB store into partition strips: one
                    # dma_start lands on one DMA engine (~11us for 256KB), so
                    # strips parallelize across engines and shrink the drain
                    # tail after the last chunk.
                    nstrip = 4 if chunk == NCHUNK - 1 else 2
                    rows = S // nstrip
                    for st in range(nstrip):
                        eng = nc.sync if st % 2 == 0 else nc.scalar
                        eng.dma_start(
                            out=out[b0 + j][st * rows : (st + 1) * rows],
                            in_=o_sb[st * rows : (st + 1) * rows],
                        )

            # Software pipeline. Per iteration the PE stream is:
            #   scores(k) | projections(k+1) | tail(k)
            # with xT DMAs for k+2 and the wT DMAs for k in flight meanwhile.
            xts = {0: load_xt(0)}
            load_weight("q", nc.scalar)
            load_weight("k", nc.scalar)
            load_biases()
            load_weight("v", nc.gpsimd)
            load_weight("o", nc.gpsimd)
            states = {0: proj(xts[0])}
            xts[1] = load_xt(1) if NCHUNK > 1 else None
            for k in range(NCHUNK):
                wt = attn_scores(states[k][0], states[k][1], pe_wt=(k == NCHUNK - 1))
                if k + 2 < NCHUNK:
                    xts[k + 2] = load_xt(k + 2)
                if k + 1 < NCHUNK:
                    states[k + 1] = proj(xts[k + 1])
                attn_tail(k, wt, states[k][2])

    nc.compile()
    return nc


def kernel(**inputs):
    if "nc" not in _CACHE:
        _CACHE["nc"] = build()
    nc = _CACHE["nc"]

    x = np.ascontiguousarray(np.asarray(inputs["x"], dtype=np.float32)).astype(
        np.float16
    )
    shared = {
        k: np.ascontiguousarray(np.asarray(inputs[k], dtype=np.float32)).astype(
            np.float16
        )
        for k in ("Wq", "Wk", "Wv", "Wo")
    }
    for k in ("bq", "bk", "bv", "bo"):
        shared[k] = np.ascontiguousarray(np.asarray(inputs[k], dtype=np.float32))
    in_maps = [
        {"x": x[i * BLOC : (i + 1) * BLOC], **shared} for i in range(NCORES)
    ]
    res = run_bass_kernel_spmd(nc, in_maps, core_ids=list(range(NCORES)))
    return np.concatenate([res.results[i]["out"] for i in range(NCORES)], axis=0)


# revision 15
# speedup vs baseline: 1.1843x; 1.1843x over previous
"""Multi-head attention (B=384, S=128, E=512, H=4, D=128) on 8 TRN2 NeuronCores.

Data-parallel: batch 384 -> 48 per core, projection weights replicated.

v2: all matmuls in fp16 (1 cyc/row at any moving size, vs fp32r's ~2-4
cyc/row at moving=128), and both transposes moved off the PE onto the DMA
XBAR (InstDmaTransposeAnt, 16-bit dtypes, ~14ns per 16x128 tile):

  xT   = transposing DMA load   x[b] DRAM [S,E] fp16 -> xt [128, EC, S] SBUF
  QT   = Wq^T @ xT + bq         [E_out, rows] fp16 (lhsT = Wq chunk, rhs = xT)
  KT   = Wk^T @ xT + bk         [E_out, rows] fp16
  V    = x @ Wv + bv            [rows, E_out] fp16 (lhsT = xT chunk, rhs = Wv)
  per batch (4 heads packed along the PSUM free dim):
    S    = qT.T @ kT            [S, H, T] scores in PSUM fp32
    w    = exp(S) fp32 (no max-sub: |S| < 88 so fp32 exp cannot overflow)
    wn   = w * (1/rowsum)       normalize -> fp16 (Pool engine)
    wT   = transposing DMA      SBUF [S, H*T] -> [T, H, S] SBUF
    attT = lhsT(v) @ wT         [D, H, S] fp16 inputs, PSUM fp32
  O    = att @ Wo + bo          (lhsT = attT chunk, rhs = Wo) -> [rows, E] f32

Precision: fp16 inputs give ~3e-4 elementwise rounding; logit abs err
~8e-3 -> output rel err ~3-4e-3 (gate 2e-2). exp/softmax stats stay fp32.

Engine split per chunk (vs ~15.4us of PE matmul): ACT q/k bias-adds + exp
(~8us), DVE v bias + reduce_sum + recip (~6us), Pool normalize + attT
copies + o bias (~8us), SP issues all DMAs (~8us). Weights go out on the
ACT DMA queue at startup so they don't serialize behind x on SP.

Scheduling: engine streams execute in emission order, software pipeline:
scores(k) | xT-DMAs(k+2) | projections(k+1) | attention-tail(k). The wT
DMA for chunk k is issued during scores(k) and consumed by the tail after
proj(k+1), hiding ~1.4us of DMA latency; the last chunk transposes wT on
the PE instead (idle at drain). Dummy bf16 matmuls warm the PE HAM
clock-gate during the initial weight/x DMA window.
"""

import numpy as np

import concourse.bass as bass
import concourse.tile as tile
import concourse.mybir as mybir
from concourse import bacc
from concourse.bass_utils import run_bass_kernel_spmd
from concourse.masks import make_identity

B, S, E, H, D = 384, 128, 512, 4, 128
NCORES = 8
BLOC = B // NCORES  # 48 batches per core
NB = 4  # batches per chunk
NCHUNK = BLOC // NB
NBS = NB * S  # 512 rows of x per chunk
EC = E // 128  # 4 chunks of the embed dim

F32 = mybir.dt.float32
F16 = mybir.dt.float16
BF16 = mybir.dt.bfloat16

_CACHE = {}


def build():
    nc = bacc.Bacc("TRN2", target_bir_lowering=False, debug=False, num_devices=NCORES)

    x = nc.dram_tensor("x", [BLOC, S, E], F16, kind="ExternalInput").ap()
    wq = nc.dram_tensor("Wq", [E, E], F16, kind="ExternalInput").ap()
    wk = nc.dram_tensor("Wk", [E, E], F16, kind="ExternalInput").ap()
    wv = nc.dram_tensor("Wv", [E, E], F16, kind="ExternalInput").ap()
    wo = nc.dram_tensor("Wo", [E, E], F16, kind="ExternalInput").ap()
    bq = nc.dram_tensor("bq", [E], F32, kind="ExternalInput").ap()
    bk = nc.dram_tensor("bk", [E], F32, kind="ExternalInput").ap()
    bv = nc.dram_tensor("bv", [E], F32, kind="ExternalInput").ap()
    bo = nc.dram_tensor("bo", [E], F32, kind="ExternalInput").ap()
    out = nc.dram_tensor("out", [BLOC, S, E], F32, kind="ExternalOutput").ap()

    with tile.TileContext(nc) as tc:
        with (
            tc.tile_pool(name="singles", bufs=1) as singles,
            tc.tile_pool(name="xp", bufs=4) as xp,
            tc.tile_pool(name="qkv", bufs=3) as qkv,
            tc.tile_pool(name="attn", bufs=3) as attn,
            tc.tile_pool(name="wsm", bufs=4) as wsm,
            tc.tile_pool(name="stats", bufs=8) as stats,
            tc.tile_pool(name="ps", bufs=8, space="PSUM") as ps,
        ):
            # --- weights / biases / identities ---
            w_sb = {}
            w_dram = {"q": wq, "k": wk, "v": wv, "o": wo}
            for name in ("q", "k", "v", "o"):
                w_sb[name] = singles.tile([128, EC, E], F16, tag=f"w{name}", name=f"w{name}")

            def load_weight(name, engine=None):
                # Weight DMAs are spread across issue queues so no engine's
                # compute stream queues behind 16 big DMA issues: q/k go out
                # on the ACT hwdge queue (needed first, few issues), v/o on
                # the gpsimd SWDGE queue (Pool is idle during startup).
                eng = engine if engine is not None else nc.scalar
                for c in range(EC):
                    eng.dma_start(
                        out=w_sb[name][:, c, :],
                        in_=w_dram[name][c * 128 : (c + 1) * 128, :],
                    )

            bq_sb = singles.tile([128, EC], F32, tag="bq")
            bk_sb = singles.tile([128, EC], F32, tag="bk")
            bv_sb = singles.tile([128, E], F32, tag="bv")
            bo_sb = singles.tile([128, E], F32, tag="bo")

            def load_biases():
                for t, b in ((bq_sb, bq), (bk_sb, bk)):
                    nc.sync.dma_start(
                        out=t,
                        in_=bass.AP(tensor=b.tensor, offset=0, ap=[[1, 128], [128, EC]]),
                    )
                for t, b in ((bv_sb, bv), (bo_sb, bo)):
                    nc.sync.dma_start(
                        out=t,
                        in_=bass.AP(tensor=b.tensor, offset=0, ap=[[0, 128], [1, E]]),
                    )

            ident_f16 = singles.tile([128, 128], F16, tag="idf16")
            make_identity(nc, ident_f16[:])

            # Warm the PE HAM clock-gate during the initial weight/x DMA
            # window with dummy matmuls (PE would otherwise start cold at
            # half clock). Output is never read.
            dummy_bf = singles.tile([128, E], BF16, tag="dummy")
            nc.vector.memset(dummy_bf, 0.0)
            ident_bf = singles.tile([128, 128], BF16, tag="idb")
            make_identity(nc, ident_bf[:])
            warm_ps = ps.tile([128, E], F32, tag="ps", name="warm")
            for _ in range(36):
                nc.tensor.matmul(warm_ps, ident_bf[:], dummy_bf, start=True, stop=True)

            def load_xt(chunk):
                """One transposing DMA per chunk: the whole 4-batch block
                [NBS, E] (DRAM-contiguous) transposes into xt [128, EC, NBS]
                with xt[p, c, j*128+s] = x[b0+j][s, c*128+p]."""
                b0 = chunk * NB
                xt = xp.tile([128, EC, NBS], F16, tag="xt")
                nc.sync.dma_start(
                    out=xt,
                    in_=bass.AP(
                        tensor=x.tensor, offset=b0 * S * E, ap=[[E, NBS], [1, E]]
                    ),
                    transpose=True,
                )
                return xt

            def proj(xt):
                """QT/KT/V projections from xT (all fp16)."""
                qt, kt = [], []
                for h in range(H):
                    p = ps.tile([128, NBS], F32, tag="ps")
                    for c in range(EC):
                        nc.tensor.matmul(
                            p,
                            w_sb["q"][:, c, h * 128 : (h + 1) * 128],
                            xt[:, c, :],
                            start=(c == 0),
                            stop=(c == EC - 1),
                        )
                    t = qkv.tile([128, NBS], F16, tag=f"qt{h}")
                    nc.scalar.add(out=t, in_=p, add=bq_sb[:, h : h + 1])
                    qt.append(t)
                    p = ps.tile([128, NBS], F32, tag="ps")
                    for c in range(EC):
                        nc.tensor.matmul(
                            p,
                            w_sb["k"][:, c, h * 128 : (h + 1) * 128],
                            xt[:, c, :],
                            start=(c == 0),
                            stop=(c == EC - 1),
                        )
                    t = qkv.tile([128, NBS], F16, tag=f"kt{h}")
                    nc.scalar.add(out=t, in_=p, add=bk_sb[:, h : h + 1])
                    kt.append(t)
                v_sb = []
                for j in range(NB):
                    p = ps.tile([128, E], F32, tag="ps")
                    for c in range(EC):
                        nc.tensor.matmul(
                            p,
                            xt[:, c, j * 128 : (j + 1) * 128],
                            w_sb["v"][:, c, :],
                            start=(c == 0),
                            stop=(c == EC - 1),
                        )
                    t = qkv.tile([128, E], F16, tag=f"v{j}")
                    nc.vector.tensor_add(out=t, in0=p, in1=bv_sb)
                    v_sb.append(t)
                return qt, kt, v_sb

            def attn_scores(qt, kt, pe_wt=False):
                """scores + softmax (no max-sub, fp32 exp/stats) -> fp16
                normalized w, transposed to wT in ONE chunk-wide DMA XBAR
                transpose (or on the PE for the drain chunk)."""
                w_f16 = wsm.tile([128, NB, H, 128], F16, tag="wf16")
                wt = wsm.tile([128, NB, H, 128], F16, tag="wt", name="wt")
                for j in range(NB):
                    ps_s = ps.tile([128, H, 128], F32, tag="ps")
                    for h in range(H):
                        nc.tensor.matmul(
                            ps_s[:, h, :],
                            qt[h][:, j * 128 : (j + 1) * 128],
                            kt[h][:, j * 128 : (j + 1) * 128],
                            start=True,
                            stop=True,
                        )
                    w_exp = wsm.tile([128, H, 128], F32, tag="wexp")
                    nc.scalar.activation(
                        out=w_exp,
                        in_=ps_s,
                        func=mybir.ActivationFunctionType.Exp,
                        bias=0.0,
                        scale=1.0,
                    )
                    sumexp = stats.tile([128, H], F32, tag="sumexp")
                    nc.vector.reduce_sum(
                        out=sumexp, in_=w_exp, axis=mybir.AxisListType.X
                    )
                    recip = stats.tile([128, H], F32, tag="recip")
                    nc.vector.reciprocal(out=recip, in_=sumexp)
                    for h in range(H):
                        nc.vector.tensor_scalar_mul(
                            out=w_f16[:, j, h, :],
                            in0=w_exp[:, h, :],
                            scalar1=recip[:, h : h + 1],
                        )
                if pe_wt:
                    for j in range(NB):
                        ps_wt = ps.tile([128, H, 128], F16, tag="ps")
                        for h in range(H):
                            nc.tensor.transpose(
                                ps_wt[:, h, :], w_f16[:, j, h, :], ident_f16[:]
                            )
                        nc.vector.tensor_copy(out=wt[:, j], in_=ps_wt)
                else:
                    nc.sync.dma_start(out=wt, in_=w_f16, transpose=True)
                return wt

            def attn_tail(chunk, wt, v_sb):
                """att = v.T-form matmuls against wT, O projection, store."""
                b0 = chunk * NB
                ats = []
                for j in range(NB):
                    ps_at = ps.tile([128, H, 128], F32, tag="ps")
                    for h in range(H):
                        nc.tensor.matmul(
                            ps_at[:, h, :],
                            v_sb[j][:, h * 128 : (h + 1) * 128],
                            wt[:, j, h, :],
                            start=True,
                            stop=True,
                        )
                    at = attn.tile([128, H, 128], F16, tag=f"at{j}")
                    if j % 2 == 0:
                        nc.scalar.copy(out=at, in_=ps_at)
                    else:
                        nc.vector.tensor_copy(out=at, in_=ps_at)
                    ats.append(at)
                for j in range(NB):
                    p = ps.tile([128, E], F32, tag="ps")
                    for h in range(H):
                        nc.tensor.matmul(
                            p,
                            ats[j][:, h, :],
                            w_sb["o"][:, h, :],
                            start=(h == 0),
                            stop=(h == H - 1),
                        )
                    o_sb = attn.tile([128, E], F32, tag=f"o{j}")
                    nc.vector.tensor_add(out=o_sb, in0=p, in1=bo_sb)
                    # Split each 256KB store into partition strips: one
                    # dma_start lands on one DMA engine (~11us for 256KB), so
                    # strips parallelize across engines and shrink the drain
                    # tail after the last chunk.
                    last = chunk == NCHUNK - 1
                    nstrip = 4 if last else 2
                    rows = S // nstrip
                    for st in range(nstrip):
                        # ACT queue only at drain (its compute stream is done);
                        # mid-stream an ACT-issued DMA stalls bias-adds/exp.
                        eng = nc.scalar if (last and st % 2 == 1) else nc.sync
                        eng.dma_start(
                            out=out[b0 + j][st * rows : (st + 1) * rows],
                            in_=o_sb[st * rows : (st + 1) * rows],
                        )

            # Software pipeline. Per iteration the PE stream is:
            #   scores(k) | projections(k+1) | tail(k)
            # with xT DMAs for k+2 and the wT DMAs for k in flight meanwhile.
            xts = {0: load_xt(0)}
            load_weight("q", nc.scalar)
            load_weight("k", nc.scalar)
            load_biases()
            load_weight("v", nc.gpsimd)
            load_weight("o", nc.gpsimd)
            states = {0: proj(xts[0])}
            xts[1] = load_xt(1) if NCHUNK > 1 else None
            for k in range(NCHUNK):
                wt = attn_scores(states[k][0], states[k][1], pe_wt=(k == NCHUNK - 1))
                if k + 2 < NCHUNK:
                    xts[k + 2] = load_xt(k + 2)
                if k + 1 < NCHUNK:
                    states[k + 1] = proj(xts[k + 1])
                attn_tail(k, wt, states[k][2])

    nc.compile()
    return nc


def kernel(**inputs):
    if "nc" not in _CACHE:
        _CACHE["nc"] = build()
    nc = _CACHE["nc"]

    x = np.ascontiguousarray(np.asarray(inputs["x"], dtype=np.float32)).astype(
        np.float16
    )
    shared = {
        k: np.ascontiguousarray(np.asarray(inputs[k], dtype=np.float32)).astype(
            np.float16
        )
        for k in ("Wq", "Wk", "Wv", "Wo")
    }
    for k in ("bq", "bk", "bv", "bo"):
        shared[k] = np.ascontiguousarray(np.asarray(inputs[k], dtype=np.float32))
    in_maps = [
        {"x": x[i * BLOC : (i + 1) * BLOC], **shared} for i in range(NCORES)
    ]
    res = run_bass_kernel_spmd(nc, in_maps, core_ids=list(range(NCORES)))
    return np.concatenate([res.results[i]["out"] for i in range(NCORES)], axis=0)
